# revision 1
# baseline (speedup 1.0000x reference)
"""GCN layer on 8 Trainium2 NeuronCores.

Computes relu(D^-1/2 (A+I) D^-1/2 X W + b) for N=8192, d=256.

Sharding: row-shard adj over N across the 8 cores (1024 rows each); x, W, b
replicated. Each core's adj shard is uploaded as the bf16 SBUF image it will
occupy on chip: partition p holds adj[1024c+i, 128k+p] at column k*1024+i,
i.e. the contraction dim j sits on partitions (as the PE matmul needs) and
every partition's data is one contiguous DRAM run (full DMA line rate).

Pipeline per core (single NEFF):
  1. Stream the 16MB shard once (HWDGE, 2MB slices) into the persistent SBUF
     cache; the tensor engine reduces row sums (matmul vs ones) as slices
     land.
  2. One AllGather ships the 8 local degree vectors (4KB each); degrees come
     back through a natural (contiguous) DMA + PE transpose into
     per-partition D^-1/2 tables.
  3. U^T = ((A+I) y)^T with y = D^-1/2 x: x chunks are scaled in place
     (Scalar/Vector engines alternating, all ahead of the matmuls), then 256
     accumulating matmuls run from SBUF; +I enters via identity-matmuls of
     the core's own y rows.
  4. Scale by own D^-1/2 (free-dim broadcast via a DMA broadcast round trip),
     apply W, bias, ReLU, and write the output block transposed; the host
     stitches the 8 blocks.
"""

import numpy as np

N = 8192
D = 256
NCORES = 8
R = N // NCORES  # rows per core = 1024
KT = N // 128  # 64 j-tiles
TS = R // 128  # 8 own-row tiles

_CACHE = {}


def _build_nc():
    import concourse.bacc as bacc
    import concourse.tile as tile
    import concourse.mybir as mybir

    f32 = mybir.dt.float32
    bf16 = mybir.dt.bfloat16
    AF = mybir.ActivationFunctionType

    nc = bacc.Bacc("TRN2", target_bir_lowering=False, debug=False,
                   num_devices=NCORES)

    adjS = nc.dram_tensor("adjS", [128, KT * R], bf16, kind="ExternalInput")
    xS = nc.dram_tensor("xS", [128, KT * D], bf16, kind="ExternalInput")
    xoS = nc.dram_tensor("xoS", [128, TS * D], bf16, kind="ExternalInput")
    Win = nc.dram_tensor("W", [D, D], bf16, kind="ExternalInput")
    bin_ = nc.dram_tensor("b", [D], f32, kind="ExternalInput")
    eyeb = nc.dram_tensor("eye", [128, 128], bf16, kind="ExternalInput")
    eyef = nc.dram_tensor("eyef", [128, 128], f32, kind="ExternalInput")
    outT = nc.dram_tensor("outT", [D, R], f32, kind="ExternalOutput")

    with tile.TileContext(nc) as tc:
        from contextlib import ExitStack

        with ExitStack() as ctx:
            pp = ctx.enter_context(tc.tile_pool(name="persist", bufs=1))
            dp = ctx.enter_context(tc.tile_pool(name="dram", bufs=1, space="DRAM"))

            # ---- persistent SBUF tensors ----
            adjTb = pp.tile([128, KT * R], bf16)   # 128KB/partition cache
            xb = pp.tile([128, KT * D], bf16)      # x, partition = j%128
            xob = pp.tile([128, TS * D], bf16)     # own x rows
            Wb = pp.tile([128, 2 * D], bf16)       # W, partition = n%128
            bsb = pp.tile([128, 2], f32)           # bias, partition = m%128
            eye_s = pp.tile([128, 128], bf16)
            eyef_s = pp.tile([128, 128], f32)
            ones_bf = pp.tile([128, 1], bf16)
            deg_s = pp.tile([1, R], f32)           # local degree (+1)
            disl = pp.tile([1, R], f32)            # local D^-1/2
            degn = pp.tile([64, 128], f32)         # gathered degrees, natural
            degln = pp.tile([8, 128], f32)         # local degrees, natural
            dis_pp = pp.tile([128, KT], f32)       # D^-1/2, partition = j%128
            diso = pp.tile([128, TS], f32)         # own D^-1/2, partition = i%128
            disrep = pp.tile([128, R], f32)        # own D^-1/2 on free dim
            y2 = [pp.tile([128, R], bf16, name=f"y2_{i}") for i in range(2)]
            outsb = [pp.tile([128, R], f32, name=f"outsb_{i}") for i in range(2)]

            degl_d = dp.tile([R], f32)
            dega_d = dp.tile([N], f32)
            disl_d = dp.tile([R], f32)

            nc.any.memset(ones_bf[:], 1.0)

            # ---- phase 1: stream the SBUF image + row sums on PE ----
            GC = 4  # j-tiles per DMA slice (1MB each, 8KB/partition runs)
            for g in range(KT // GC):
                c0, c1 = g * GC * R, (g + 1) * GC * R
                nc.sync.dma_start(out=adjTb[:, c0:c1], in_=adjS.ap()[:, c0:c1])
            # small loads after the degree-critical stream; all are consumed
            # only once the collective completes (~30us later).
            nc.sync.dma_start(out=eyef_s[:, :], in_=eyef.ap())
            nc.sync.dma_start(out=xob[:, :], in_=xoS.ap())
            nc.sync.dma_start(
                out=Wb[:, :].rearrange("p (k m) -> p k m", m=D),
                in_=Win.ap().rearrange("(k p) m -> p k m", p=128))
            nc.sync.dma_start(
                out=bsb[:, :], in_=bin_.ap().rearrange("(h p) -> p h", p=128))
            nc.sync.dma_start(out=eye_s[:, :], in_=eyeb.ap())
            nc.sync.dma_start(out=xb[:, :], in_=xS.ap())

            pdeg = ctx.enter_context(
                tc.tile_pool(name="psdeg", bufs=1, space="PSUM"))
            pst = ctx.enter_context(
                tc.tile_pool(name="pst", bufs=1, space="PSUM"))
            psuo = ctx.enter_context(
                tc.tile_pool(name="psuo", bufs=2, space="PSUM"))

            dps = pdeg.tile([1, 1024], f32, padded_shape=[128, 1024])
            for k in range(KT):
                for s in range(2):
                    nc.tensor.matmul(
                        dps[:, s * 512:(s + 1) * 512], ones_bf[:, :],
                        adjTb[:, k * R + s * 512:k * R + (s + 1) * 512],
                        start=(k == 0), stop=(k == KT - 1),
                        skip_group_check=True)
            # deg = rowsum + 1 (the +I term)
            for s in range(2):
                nc.vector.tensor_scalar_add(
                    deg_s[:, s * 512:(s + 1) * 512],
                    dps[:, s * 512:(s + 1) * 512], 1.0)

            # ---- phase 2: AllGather degrees ----
            nc.scalar.dma_start(out=degl_d[:], in_=deg_s[0:1, :])
            nc.gpsimd.collective_compute(
                "AllGather", mybir.AluOpType.bypass,
                replica_groups=[list(range(NCORES))],
                ins=[degl_d.opt()], outs=[dega_d.opt()])

            # gathered degrees -> per-partition D^-1/2 via PE transpose
            tall = pst.tile([128, 72], f32)
            nc.scalar.dma_start(
                out=degn[:, :], in_=dega_d.opt().rearrange("(c f) -> c f", f=128))
            nc.tensor.transpose(tall[:, 0:64], degn[:, :], eyef_s[0:64, 0:64])
            nc.vector.reciprocal_approx_fast(dis_pp[:, :], tall[:, 0:64])
            nc.scalar.activation(dis_pp[:, :], dis_pp[:, :], AF.Sqrt)
            # local degrees -> own D^-1/2 table (for the +I rows)
            nc.scalar.dma_start(
                out=degln[:, :], in_=degl_d.opt().rearrange("(c f) -> c f", f=128))
            nc.tensor.transpose(tall[:, 64:72], degln[:, :], eyef_s[0:8, 0:8])
            nc.vector.reciprocal_approx_fast(diso[:, :], tall[:, 64:72])
            nc.scalar.activation(diso[:, :], diso[:, :], AF.Sqrt)

            # local dis for the free-dim broadcast (via DRAM round trip);
            # emitted after the collective so its slow Sqrt-table load and
            # DMAs don't share a semaphore group with the trigger.
            nc.vector.reciprocal_approx_fast(disl[:, :], deg_s[:, :])
            nc.scalar.activation(disl[:, :], disl[:, :], AF.Sqrt)
            nc.scalar.dma_start(out=disl_d[:], in_=disl[0:1, :])
            nc.scalar.dma_start(
                out=disrep[:, :],
                in_=disl_d.opt().unsqueeze(0).partition_broadcast(128))

            # ---- phase 3: y = dis*x, then U^T = ((A+I) y)^T ----
            u = [psuo.tile([128, R], f32, name=f"u_{i}", tag="uo")
                 for i in range(2)]

            def scale_y(k):
                chunk = xb[:, k * D:(k + 1) * D]
                if k % 2 == 0:
                    nc.scalar.activation(chunk, chunk, AF.Copy,
                                         scale=dis_pp[:, k:k + 1])
                else:
                    nc.vector.tensor_scalar_mul(chunk, chunk,
                                                dis_pp[:, k:k + 1])

            for k in range(KT):
                scale_y(k)
            for k in range(KT):
                for h in range(2):
                    for s in range(2):
                        nc.tensor.matmul(
                            u[h][:, s * 512:(s + 1) * 512],
                            xb[:, k * D + h * 128:k * D + (h + 1) * 128],
                            adjTb[:, k * R + s * 512:k * R + (s + 1) * 512],
                            start=(k == 0), stop=False,
                            skip_group_check=True)
            # +I: U^T[n, own block t] += y_own[t]^T
            for t in range(TS):
                chunk = xob[:, t * D:(t + 1) * D]
                nc.scalar.activation(chunk, chunk, AF.Copy,
                                     scale=diso[:, t:t + 1])
                for h in range(2):
                    nc.tensor.matmul(
                        u[h][:, t * 128:(t + 1) * 128],
                        xob[:, t * D + h * 128:t * D + (h + 1) * 128],
                        eye_s[:, :],
                        start=False, stop=(t == TS - 1),
                        skip_group_check=True)

            # ---- phase 4: scale columns by own dis, cast to bf16 ----
            for h in range(2):
                nc.vector.tensor_mul(y2[h][:, :], u[h][:, :], disrep[:, :])

            # ---- phase 5: out^T = W^T @ (scaled U^T) ----
            o = [psuo.tile([128, R], f32, name=f"o_{i}", tag="uo")
                 for i in range(2)]
            for mh in range(2):
                for nk in range(2):
                    for s in range(2):
                        nc.tensor.matmul(
                            o[mh][:, s * 512:(s + 1) * 512],
                            Wb[:, nk * D + mh * 128:nk * D + (mh + 1) * 128],
                            y2[nk][:, s * 512:(s + 1) * 512],
                            start=(nk == 0), stop=(nk == 1),
                            skip_group_check=True)

            # ---- phase 6: relu(out^T + b), write transposed output ----
            for mh in range(2):
                nc.scalar.activation(
                    outsb[mh][:, :], o[mh][:, :], AF.Relu,
                    bias=bsb[:, mh:mh + 1], scale=1.0)
                nc.sync.dma_start(
                    out=outT.ap()[mh * 128:(mh + 1) * 128, :],
                    in_=outsb[mh][:, :])

    nc.compile()
    return nc


def _get_nc():
    if "nc" not in _CACHE:
        _CACHE["nc"] = _build_nc()
    return _CACHE["nc"]


def _sbuf_image(mat_bf16):
    """[T*128, F] -> [128, T*F] where partition p holds rows {128t+p}."""
    t128, f = mat_bf16.shape
    t = t128 // 128
    return np.ascontiguousarray(
        mat_bf16.reshape(t, 128, f).transpose(1, 0, 2).reshape(128, t * f))


def kernel(x, adj, W, b):
    import ml_dtypes
    from concourse.bass_utils import run_bass_kernel_spmd

    bf = ml_dtypes.bfloat16
    x = np.asarray(x, dtype=np.float32)
    adj = np.asarray(adj, dtype=np.float32)
    W = np.ascontiguousarray(np.asarray(W, dtype=np.float32)).astype(bf)
    b = np.ascontiguousarray(np.asarray(b, dtype=np.float32))

    nc = _get_nc()

    x_bf = np.ascontiguousarray(x).astype(bf)
    xS = _sbuf_image(x_bf)
    eye_np = np.eye(128, dtype=bf)
    eyef_np = np.eye(128, dtype=np.float32)
    in_maps = []
    for c in range(NCORES):
        rows = slice(c * R, (c + 1) * R)
        adjT_c = np.ascontiguousarray(adj[rows, :].T).astype(bf)
        in_maps.append({
            "adjS": _sbuf_image(adjT_c),
            "xS": xS,
            "xoS": _sbuf_image(x_bf[rows, :]),
            "W": W,
            "b": b,
            "eye": eye_np,
            "eyef": eyef_np,
        })

    res = run_bass_kernel_spmd(nc, in_maps, core_ids=list(range(NCORES)))
    out = np.concatenate(
        [np.asarray(res.results[c]["outT"]).T for c in range(NCORES)], axis=0)
    return np.ascontiguousarray(out, dtype=np.float32)


if __name__ == "__main__":
    rng = np.random.default_rng(0)
    x = rng.standard_normal((N, D)).astype(np.float32)
    adj = rng.random((N, N)).astype(np.float32)
    W = rng.standard_normal((D, D)).astype(np.float32) * 0.06
    b = rng.standard_normal((D,)).astype(np.float32) * 0.06
    out = kernel(x=x, adj=adj, W=W, b=b)
    print(out.shape, out.dtype)

